# revision 13
# baseline (speedup 1.0000x reference)
"""Trainium2 Bass kernel for nn_Conv2d_20590073217670.

Conv2d: input [32,64,64,64] (NCHW), weight [576,128] (unfold layout:
row = ci*9 + a*3 + b for tap (a,b)), bias [1,128,1,1], stride 1, pad 1.
Output [32,128,64,64].

Strategy: data-parallel over batch — 4 images per NeuronCore, 8 cores.
The host pre-casts the input to bf16 and pre-pads it into
xp[n, c, t, j] = img[n, c, t-1, j-1] (zero border rows/cols); the
device builds two padded [128, 66, *] GEMM layouts:
  xb: parts 0:64  <- DMA xp[n]           (= img[c, r-1, j-1])
      parts 64:128 <- DMA xp[n][:,1:65]  (= img[c, r,   j-1], one row up)
  xc: parts 0:64  <- DVE copy of xb lower
      parts 64:128 <- Act copy of xb lower shifted one col left
                                         (= img[c, r-1, j  ])
Per 8-row output block, 5 matmul passes, each a uniform full [8, 64]
PSUM tile (border taps read the zero padding):
  3x K=128: vertical tap pairs (0,b)+(1,b) from xb        (b = 0,1,2)
  1x K=128: horizontal tap pair (2,0)+(2,1) from xc at +2 rows
  1x K=64 : tap (2,2) from xb lower at +2 rows, col 2
Four blocks are processed pass-major so consecutive matmuls rotate
over 4 PSUM banks and pipeline through the PE array at ~216ns each.

Perf notes (from NTFF traces):
  - The core starts under a 50% utilization throttle and ramps to 100%
    only after ~5us of sustained PE activity; matmuls run at ~427ns
    instead of 216ns until then.  A chain of small warm-up matmuls on
    memset scratch is issued during the (otherwise PE-idle) preamble +
    input-load window so the ramp completes before the real stream.
  - Output is bf16 on device (half the outbound DMA bytes); the host
    casts back to f32.  PSUM eviction fuses bias add + down-cast.
  - Weights are five per-pass [128,128] slabs interleaved with image
    0's input chunks across both HWDGE rings so nothing stalls the
    first passes; image 0 is chunked fine-grained for an early start.
  - The last image's outputs are written per 16 rows alternating
    between the two rings to shorten the drain.
"""
import sys

for _p in ("/opt/trn_rl_repo", "/root/.axon_site/_ro/trn_rl_repo"):
    if _p not in sys.path:
        sys.path.append(_p)

import numpy as np
import ml_dtypes
from contextlib import ExitStack

import concourse.bacc as bacc
import concourse.tile as tile
from concourse import mybir
from concourse.bass_utils import run_bass_kernel_spmd

f32 = mybir.dt.float32
bf16 = mybir.dt.bfloat16

N_CORES = 8
NB = 4  # images per core
N_WARM = 0  # PE warm-up disabled: full-K dummies trip the sustained-power
            # cap (whole stream drops to ~84% rate), K=1 dummies don't feed
            # the utilization-limiter ramp at all


def build_nc():
    nc = bacc.Bacc()
    xp = nc.declare_dram_parameter("xp", [NB, 64, 66, 66], bf16, isOutput=False)
    wph = nc.declare_dram_parameter("wph", [128, 5, 128], bf16, isOutput=False)
    bias = nc.declare_dram_parameter("b", [128, 1], f32, isOutput=False)
    out = nc.declare_dram_parameter("out", [NB, 128, 64, 64], bf16, isOutput=True)

    with tile.TileContext(nc) as tc, ExitStack() as ctx:
        const = ctx.enter_context(tc.tile_pool(name="const", bufs=1))
        xb_pool = ctx.enter_context(tc.tile_pool(name="xb", bufs=3))
        xc_pool = ctx.enter_context(tc.tile_pool(name="xc", bufs=2))
        ob_pool = ctx.enter_context(tc.tile_pool(name="ob", bufs=2))
        ps_pool = ctx.enter_context(tc.tile_pool(name="ps", bufs=2, space="PSUM"))

        if N_WARM:
            # PE warm-up: small matmuls on memset scratch rotating over this
            # generation's 4 PSUM banks, holding PE activity high while the
            # hardware utilization limiter ramps 50% -> 100%.
            wsrc = const.tile([128, 4, 64], bf16)
            nc.vector.memset(wsrc[:], 0.0)
            # reuse the P0..P3 slot names so the pool footprint stays 8 banks
            D = [ps_pool.tile([128, 8, 64], f32, name=f"P{i}") for i in range(4)]
            for i in range(N_WARM):
                nc.tensor.matmul(
                    D[i % 4][:, 0:4, :], wsrc[0:1, 0:2, :], wsrc[0:1, :, :],
                    start=True, stop=True,
                )

        # ---- weights, pre-arranged bf16 on the host as 5 per-pass slabs
        # wph[p] is the [128,128] stationary for pass p:
        #   p=0..2: lower = tap (0,p), upper = tap (1,p)
        #   p=3:    lower = tap (2,0), upper = tap (2,1)
        #   p=4:    lower = tap (2,2), upper = zeros (pass runs K=64)
        wt = const.tile([128, 5, 128], bf16)
        bt = const.tile([128, 1], f32)

        def emit_image_dmas(n):
            """DMA xp[n] into a fresh xb tile (lower + row-shifted upper).
            Image 0 is chunked: lower rides the sync ring behind the single
            weight load, upper gets the scalar ring to itself, so the first
            matmuls start as early as both rings allow.  The bias (tiny but
            128 four-byte packets) goes last - it is only needed by the
            first eviction."""
            xb = xb_pool.tile([128, 66, 66], bf16)
            if n == 0:
                nc.scalar.dma_start(out=wt[:], in_=wph[:])
                nc.sync.dma_start(out=xb[0:64, 0:10, :], in_=xp[0][:, 0:10, :])
                nc.sync.dma_start(out=xb[0:64, 10:35, :], in_=xp[0][:, 10:35, :])
                nc.sync.dma_start(out=xb[0:64, 35:66, :], in_=xp[0][:, 35:66, :])
                nc.scalar.dma_start(out=xb[64:128, 0:9, :], in_=xp[0][:, 1:10, :])
                nc.scalar.dma_start(out=xb[64:128, 9:34, :], in_=xp[0][:, 10:35, :])
                nc.scalar.dma_start(out=xb[64:128, 34:64, :], in_=xp[0][:, 35:65, :])
                nc.sync.dma_start(out=bt[:], in_=bias[:])
            else:
                nc.sync.dma_start(out=xb[0:64, :, :], in_=xp[n][:, :, :])
                nc.sync.dma_start(out=xb[64:128, 0:64, :], in_=xp[n][:, 1:65, :])
            return xb

        def emit_image_copies(n, xb):
            """Derive xc from xb: lower = xb lower; upper = one col left
            (img[c, r-1, j]).  Only rows 2:66 / cols 0:65 are ever read."""
            xc = xc_pool.tile([128, 66, 65], bf16)
            if n == 0:
                nc.vector.tensor_copy(xc[0:64, 2:35, :], xb[0:64, 2:35, 0:65])
                nc.vector.tensor_copy(xc[64:128, 2:35, :], xb[0:64, 2:35, 1:66])
                nc.scalar.copy(xc[0:64, 35:66, :], xb[0:64, 35:66, 0:65])
                nc.scalar.copy(xc[64:128, 35:66, :], xb[0:64, 35:66, 1:66])
            else:
                nc.vector.tensor_copy(xc[0:64, 2:66, :], xb[0:64, 2:66, 0:65])
                nc.scalar.copy(xc[64:128, 2:66, :], xb[0:64, 2:66, 1:66])
            return xc

        xb_cur = emit_image_dmas(0)
        tiles = (xb_cur, emit_image_copies(0, xb_cur))
        for n in range(NB):
            xb, xc = tiles
            if n + 1 < NB:
                # issue next image's input DMAs now: they get maximum lead
                # on the sync ring (which carries only input in steady state)
                xb_next = emit_image_dmas(n + 1)

            osb = ob_pool.tile([128, 64, 64], bf16)
            for half in range(2):
                # pass-major over 4 blocks: consecutive matmuls rotate over 4
                # PSUM banks, pipelining the PE and reusing each weight 4x
                P0 = ps_pool.tile([128, 8, 64], f32)
                P1 = ps_pool.tile([128, 8, 64], f32)
                P2 = ps_pool.tile([128, 8, 64], f32)
                P3 = ps_pool.tile([128, 8, 64], f32)
                Ps = (P0, P1, P2, P3)
                ys = [half * 32 + q * 8 for q in range(4)]
                for p in range(5):
                    st, sp = (p == 0), (p == 4)
                    for P, y0 in zip(Ps, ys):
                        if p < 3:
                            nc.tensor.matmul(
                                P[:, :, :], wt[:, p, :],
                                xb[:, y0:y0 + 8, p:p + 64],
                                start=st, stop=sp,
                            )
                        elif p == 3:
                            nc.tensor.matmul(
                                P[:, :, :], wt[:, 3, :],
                                xc[:, y0 + 2:y0 + 10, 0:64],
                                start=st, stop=sp,
                            )
                        else:
                            nc.tensor.matmul(
                                P[:, :, :], wt[0:64, 4, :],
                                xb[0:64, y0 + 2:y0 + 10, 2:66],
                                start=st, stop=sp,
                            )
                # fused bias add + f32->bf16 down-cast, split DVE/Act
                last = n == NB - 1 and half == 1
                for q, (P, y0) in enumerate(zip(Ps, ys)):
                    dst = osb[:, y0:y0 + 8, :]
                    if last:
                        # tail: split each eviction row-wise across both
                        # engines so the drain is as short as possible
                        nc.vector.tensor_scalar_add(
                            osb[:, y0:y0 + 4, :], P[:, 0:4, :], bt[:])
                        nc.scalar.add(
                            osb[:, y0 + 4:y0 + 8, :], P[:, 4:8, :], bt[:])
                    elif q % 2 == 1:
                        nc.scalar.add(dst, P[:, :, :], bt[:])
                    else:
                        nc.vector.tensor_scalar_add(dst, P[:, :, :], bt[:])
                y0 = half * 32
                if n == NB - 1 and half == 1:
                    # tail: 8-row pieces on both rings as evictions land
                    for q in range(4):
                        eng = nc.scalar if q % 2 == 0 else nc.sync
                        eng.dma_start(
                            out=out[n][:, y0 + q * 8:y0 + q * 8 + 8, :],
                            in_=osb[:, y0 + q * 8:y0 + q * 8 + 8, :])
                elif n == NB - 1:
                    nc.scalar.dma_start(
                        out=out[n][:, y0:y0 + 16, :], in_=osb[:, y0:y0 + 16, :])
                    nc.sync.dma_start(
                        out=out[n][:, y0 + 16:y0 + 32, :],
                        in_=osb[:, y0 + 16:y0 + 32, :])
                else:
                    nc.scalar.dma_start(
                        out=out[n][:, y0:y0 + 32, :], in_=osb[:, y0:y0 + 32, :])
                if half == 0 and n + 1 < NB:
                    # next image's xc copies run on DVE/Act during this
                    # image's second half of matmuls
                    tiles = (xb_next, emit_image_copies(n + 1, xb_next))

    nc.finalize()
    return nc


_NC = None


def _get_nc():
    global _NC
    if _NC is None:
        _NC = build_nc()
    return _NC


def host_prep(inputs):
    x = np.asarray(inputs["input"], dtype=np.float32)
    w = np.asarray(inputs["weight"], dtype=np.float32)
    b = np.ascontiguousarray(
        np.asarray(inputs["bias"], dtype=np.float32).reshape(128, 1))
    # host-side bf16 cast + zero padding: xp[n, c, t, j] = x[n, c, t-1, j-1]
    N = x.shape[0]
    xp = np.zeros((N, 64, 66, 66), dtype=ml_dtypes.bfloat16)
    xp[:, :, 1:65, 1:65] = x.astype(ml_dtypes.bfloat16)
    # per-pass weight slabs (see build_nc)
    w3 = w.reshape(64, 9, 128).astype(ml_dtypes.bfloat16)
    wph = np.zeros((5, 128, 128), dtype=ml_dtypes.bfloat16)
    for p in range(3):
        wph[p, 0:64] = w3[:, p]
        wph[p, 64:128] = w3[:, 3 + p]
    wph[3, 0:64] = w3[:, 6]
    wph[3, 64:128] = w3[:, 7]
    wph[4, 0:64] = w3[:, 8]
    # device loads the weights as one [128, 5*128] DMA (contiguous per
    # partition), so transpose to partition-major
    wph = np.ascontiguousarray(wph.transpose(1, 0, 2))
    return xp, wph, b


def kernel(**inputs) -> np.ndarray:
    xp, wph, b = host_prep(inputs)
    nc = _get_nc()
    in_maps = [
        {"xp": xp[c * NB:(c + 1) * NB], "wph": wph, "b": b}
        for c in range(N_CORES)
    ]
    res = run_bass_kernel_spmd(nc, in_maps, list(range(N_CORES)))
    return np.concatenate(
        [np.asarray(r["out"], dtype=np.float32) for r in res.results], axis=0)


# revision 14
# speedup vs baseline: 1.0650x; 1.0650x over previous
"""Trainium2 Bass kernel for nn_Conv2d_20590073217670.

Conv2d: input [32,64,64,64] (NCHW), weight [576,128] (unfold layout:
row = ci*9 + a*3 + b for tap (a,b)), bias [1,128,1,1], stride 1, pad 1.
Output [32,128,64,64].

Strategy: data-parallel over batch — 4 images per NeuronCore, 8 cores.
The host pre-casts the input to bf16 and pre-pads it into
xp[n, c, t, j] = img[n, c, t-1, j-1] (zero border rows/cols); the
device builds two padded [128, 66, *] GEMM layouts:
  xb: parts 0:64  <- DMA xp[n]           (= img[c, r-1, j-1])
      parts 64:128 <- DMA xp[n][:,1:65]  (= img[c, r,   j-1], one row up)
  xc: parts 0:64  <- DVE copy of xb lower
      parts 64:128 <- Act copy of xb lower shifted one col left
                                         (= img[c, r-1, j  ])
Per 8-row output block, 5 matmul passes, each a uniform full [8, 64]
PSUM tile (border taps read the zero padding):
  3x K=128: vertical tap pairs (0,b)+(1,b) from xb        (b = 0,1,2)
  1x K=128: horizontal tap pair (2,0)+(2,1) from xc at +2 rows
  1x K=64 : tap (2,2) from xb lower at +2 rows, col 2
Four blocks are processed pass-major so consecutive matmuls rotate
over 4 PSUM banks and pipeline through the PE array at ~216ns each.

Perf notes (from NTFF traces):
  - The core starts under a 50% utilization throttle and ramps to 100%
    only after ~5us of sustained PE activity; matmuls run at ~427ns
    instead of 216ns until then.  A chain of small warm-up matmuls on
    memset scratch is issued during the (otherwise PE-idle) preamble +
    input-load window so the ramp completes before the real stream.
  - Output is bf16 on device (half the outbound DMA bytes); the host
    casts back to f32.  PSUM eviction fuses bias add + down-cast.
  - Weights are five per-pass [128,128] slabs interleaved with image
    0's input chunks across both HWDGE rings so nothing stalls the
    first passes; image 0 is chunked fine-grained for an early start.
  - The last image's outputs are written per 16 rows alternating
    between the two rings to shorten the drain.
"""
import sys

for _p in ("/opt/trn_rl_repo", "/root/.axon_site/_ro/trn_rl_repo"):
    if _p not in sys.path:
        sys.path.append(_p)

import numpy as np
import ml_dtypes
from contextlib import ExitStack

import concourse.bacc as bacc
import concourse.tile as tile
from concourse import mybir
from concourse.bass_utils import run_bass_kernel_spmd

f32 = mybir.dt.float32
bf16 = mybir.dt.bfloat16

N_CORES = 8
NB = 4  # images per core
N_WARM = 0  # PE warm-up disabled: full-K dummies trip the sustained-power
            # cap (whole stream drops to ~84% rate), K=1 dummies don't feed
            # the utilization-limiter ramp at all


def build_nc():
    nc = bacc.Bacc()
    xp = nc.declare_dram_parameter("xp", [NB, 64, 66, 66], bf16, isOutput=False)
    wph = nc.declare_dram_parameter("wph", [128, 5, 128], bf16, isOutput=False)
    bias = nc.declare_dram_parameter("b", [128, 1], f32, isOutput=False)
    out = nc.declare_dram_parameter("out", [NB, 128, 64, 64], bf16, isOutput=True)

    with tile.TileContext(nc) as tc, ExitStack() as ctx:
        const = ctx.enter_context(tc.tile_pool(name="const", bufs=1))
        xb_pool = ctx.enter_context(tc.tile_pool(name="xb", bufs=3))
        xc_pool = ctx.enter_context(tc.tile_pool(name="xc", bufs=2))
        ob_pool = ctx.enter_context(tc.tile_pool(name="ob", bufs=2))
        ps_pool = ctx.enter_context(tc.tile_pool(name="ps", bufs=2, space="PSUM"))

        if N_WARM:
            # PE warm-up: small matmuls on memset scratch rotating over this
            # generation's 4 PSUM banks, holding PE activity high while the
            # hardware utilization limiter ramps 50% -> 100%.
            wsrc = const.tile([128, 4, 64], bf16)
            nc.vector.memset(wsrc[:], 0.0)
            # reuse the P0..P3 slot names so the pool footprint stays 8 banks
            D = [ps_pool.tile([128, 8, 64], f32, name=f"P{i}") for i in range(4)]
            for i in range(N_WARM):
                nc.tensor.matmul(
                    D[i % 4][:, 0:4, :], wsrc[0:1, 0:2, :], wsrc[0:1, :, :],
                    start=True, stop=True,
                )

        # ---- weights, pre-arranged bf16 on the host as 5 per-pass slabs
        # wph[p] is the [128,128] stationary for pass p:
        #   p=0..2: lower = tap (0,p), upper = tap (1,p)
        #   p=3:    lower = tap (2,0), upper = tap (2,1)
        #   p=4:    lower = tap (2,2), upper = zeros (pass runs K=64)
        wt = const.tile([128, 5, 128], bf16)
        bt = const.tile([128, 1], f32)

        def emit_image_dmas(n):
            """DMA xp[n] into a fresh xb tile (lower + row-shifted upper).
            Image 0 is chunked: lower rides the sync ring behind the single
            weight load, upper gets the scalar ring to itself, so the first
            matmuls start as early as both rings allow.  The bias (tiny but
            128 four-byte packets) goes last - it is only needed by the
            first eviction."""
            xb = xb_pool.tile([128, 66, 66], bf16)
            if n == 0:
                nc.sync.dma_start(out=wt[:], in_=wph[:])
                nc.sync.dma_start(out=xb[0:64, 0:10, :], in_=xp[0][:, 0:10, :])
                nc.sync.dma_start(out=xb[0:64, 10:35, :], in_=xp[0][:, 10:35, :])
                nc.sync.dma_start(out=xb[0:64, 35:66, :], in_=xp[0][:, 35:66, :])
                nc.scalar.dma_start(out=xb[64:128, 0:9, :], in_=xp[0][:, 1:10, :])
                nc.scalar.dma_start(out=xb[64:128, 9:34, :], in_=xp[0][:, 10:35, :])
                nc.scalar.dma_start(out=xb[64:128, 34:64, :], in_=xp[0][:, 35:65, :])
                nc.sync.dma_start(out=bt[:], in_=bias[:])
            else:
                nc.sync.dma_start(out=xb[0:64, :, :], in_=xp[n][:, :, :])
                nc.sync.dma_start(out=xb[64:128, 0:64, :], in_=xp[n][:, 1:65, :])
            return xb

        def emit_image_copies(n, xb):
            """Derive xc from xb: lower = xb lower; upper = one col left
            (img[c, r-1, j]).  Only rows 2:66 / cols 0:65 are ever read."""
            xc = xc_pool.tile([128, 66, 65], bf16)
            if n == 0:
                nc.vector.tensor_copy(xc[0:64, 2:35, :], xb[0:64, 2:35, 0:65])
                nc.vector.tensor_copy(xc[64:128, 2:35, :], xb[0:64, 2:35, 1:66])
                nc.scalar.copy(xc[0:64, 35:66, :], xb[0:64, 35:66, 0:65])
                nc.scalar.copy(xc[64:128, 35:66, :], xb[0:64, 35:66, 1:66])
            else:
                nc.vector.tensor_copy(xc[0:64, 2:66, :], xb[0:64, 2:66, 0:65])
                nc.scalar.copy(xc[64:128, 2:66, :], xb[0:64, 2:66, 1:66])
            return xc

        xb_cur = emit_image_dmas(0)
        tiles = (xb_cur, emit_image_copies(0, xb_cur))
        for n in range(NB):
            xb, xc = tiles
            if n + 1 < NB:
                # issue next image's input DMAs now: they get maximum lead
                # on the sync ring (which carries only input in steady state)
                xb_next = emit_image_dmas(n + 1)

            osb = ob_pool.tile([128, 64, 64], bf16)
            for half in range(2):
                # pass-major over 4 blocks: consecutive matmuls rotate over 4
                # PSUM banks, pipelining the PE and reusing each weight 4x
                P0 = ps_pool.tile([128, 8, 64], f32)
                P1 = ps_pool.tile([128, 8, 64], f32)
                P2 = ps_pool.tile([128, 8, 64], f32)
                P3 = ps_pool.tile([128, 8, 64], f32)
                Ps = (P0, P1, P2, P3)
                ys = [half * 32 + q * 8 for q in range(4)]
                for p in range(5):
                    st, sp = (p == 0), (p == 4)
                    for P, y0 in zip(Ps, ys):
                        if p < 3:
                            nc.tensor.matmul(
                                P[:, :, :], wt[:, p, :],
                                xb[:, y0:y0 + 8, p:p + 64],
                                start=st, stop=sp,
                            )
                        elif p == 3:
                            nc.tensor.matmul(
                                P[:, :, :], wt[:, 3, :],
                                xc[:, y0 + 2:y0 + 10, 0:64],
                                start=st, stop=sp,
                            )
                        else:
                            nc.tensor.matmul(
                                P[:, :, :], wt[0:64, 4, :],
                                xb[0:64, y0 + 2:y0 + 10, 2:66],
                                start=st, stop=sp,
                            )
                # fused bias add + f32->bf16 down-cast, split DVE/Act
                last = n == NB - 1 and half == 1
                for q, (P, y0) in enumerate(zip(Ps, ys)):
                    dst = osb[:, y0:y0 + 8, :]
                    if last:
                        # tail: split each eviction row-wise across both
                        # engines so the drain is as short as possible
                        nc.vector.tensor_scalar_add(
                            osb[:, y0:y0 + 4, :], P[:, 0:4, :], bt[:])
                        nc.scalar.add(
                            osb[:, y0 + 4:y0 + 8, :], P[:, 4:8, :], bt[:])
                    elif q % 2 == 1:
                        nc.scalar.add(dst, P[:, :, :], bt[:])
                    else:
                        nc.vector.tensor_scalar_add(dst, P[:, :, :], bt[:])
                y0 = half * 32
                if n == NB - 1 and half == 1:
                    # tail: 8-row pieces on both rings as evictions land
                    for q in range(4):
                        eng = nc.scalar if q % 2 == 0 else nc.sync
                        eng.dma_start(
                            out=out[n][:, y0 + q * 8:y0 + q * 8 + 8, :],
                            in_=osb[:, y0 + q * 8:y0 + q * 8 + 8, :])
                elif n == NB - 1:
                    nc.scalar.dma_start(
                        out=out[n][:, y0:y0 + 16, :], in_=osb[:, y0:y0 + 16, :])
                    nc.sync.dma_start(
                        out=out[n][:, y0 + 16:y0 + 32, :],
                        in_=osb[:, y0 + 16:y0 + 32, :])
                else:
                    nc.scalar.dma_start(
                        out=out[n][:, y0:y0 + 32, :], in_=osb[:, y0:y0 + 32, :])
                if half == 0 and n + 1 < NB:
                    # next image's xc copies run on DVE/Act during this
                    # image's second half of matmuls
                    tiles = (xb_next, emit_image_copies(n + 1, xb_next))

    nc.finalize()
    return nc


_NC = None


def _get_nc():
    global _NC
    if _NC is None:
        _NC = build_nc()
    return _NC


def host_prep(inputs):
    x = np.asarray(inputs["input"], dtype=np.float32)
    w = np.asarray(inputs["weight"], dtype=np.float32)
    b = np.ascontiguousarray(
        np.asarray(inputs["bias"], dtype=np.float32).reshape(128, 1))
    # host-side bf16 cast + zero padding: xp[n, c, t, j] = x[n, c, t-1, j-1]
    N = x.shape[0]
    xp = np.zeros((N, 64, 66, 66), dtype=ml_dtypes.bfloat16)
    xp[:, :, 1:65, 1:65] = x.astype(ml_dtypes.bfloat16)
    # per-pass weight slabs (see build_nc)
    w3 = w.reshape(64, 9, 128).astype(ml_dtypes.bfloat16)
    wph = np.zeros((5, 128, 128), dtype=ml_dtypes.bfloat16)
    for p in range(3):
        wph[p, 0:64] = w3[:, p]
        wph[p, 64:128] = w3[:, 3 + p]
    wph[3, 0:64] = w3[:, 6]
    wph[3, 64:128] = w3[:, 7]
    wph[4, 0:64] = w3[:, 8]
    # device loads the weights as one [128, 5*128] DMA (contiguous per
    # partition), so transpose to partition-major
    wph = np.ascontiguousarray(wph.transpose(1, 0, 2))
    return xp, wph, b


def kernel(**inputs) -> np.ndarray:
    xp, wph, b = host_prep(inputs)
    nc = _get_nc()
    in_maps = [
        {"xp": xp[c * NB:(c + 1) * NB], "wph": wph, "b": b}
        for c in range(N_CORES)
    ]
    res = run_bass_kernel_spmd(nc, in_maps, list(range(N_CORES)))
    return np.concatenate(
        [np.asarray(r["out"], dtype=np.float32) for r in res.results], axis=0)
